# revision 13
# baseline (speedup 1.0000x reference)
"""AlgebraicTransformerBlock on 8 Trainium2 NeuronCores (Bass/Tile).

Sharding: SPMD, one program on all 8 cores. Core c owns batch b = c//4 and
the 512-token query block qb = c%4 of that batch. No collectives: each core
redundantly computes LN1 + K/V projections for its batch's full 2048 tokens
(needed for causal attention), then attention / out-proj / LN2 / FFN for its
own 512 queries, and returns a [1024, 512] feature-major output slice that
the host transposes and concatenates.

Device layout is feature-major ([d_model, tokens]) throughout:
 - LN mean / E[x^2] are computed with an all-ones stationary matmul, which
   leaves the per-token stats replicated across all 128 partitions — exactly
   the broadcast form the normalize step needs. No transposes anywhere.
 - LN gamma/beta and the attention 1/sqrt(dh) are folded into the weights
   host-side, so the device only computes (x - mean) * p(var)/q(var).
 - The causal mask and the Toeplitz relative bias are folded into one
   per-core [16, 128, 2432] bf16 "slab" (bias value where visible, -1e9
   where masked): score tile (head h, k-tile kj) adds slab[h][:, 1920-128*kj
   :][:, :512]. The reference's `w = relu(s)+1e-6 (masked)` becomes one
   fused DVE op: P = relu(S + slab) + meps, with meps the 1e-6*mask slab.
 - V is produced token-major [k, 16, 65] with column 64 of each head set to
   1.0, so the context matmul P^T-accumulation also yields the row-sums
   needed for normalization; the per-query reciprocal is applied per head
   (partition-broadcast), exactly preserving the reference epsilon algebra.

The host-side `kernel()` folds/casts weights (bf16), builds the slabs, runs
the program via run_bass_kernel_spmd, and reassembles the full [2, 2048,
1024] fp32 output. A numpy fallback reproduces the exact math if the device
path is unavailable.
"""

import sys
import types

import numpy as np

D_MODEL, N_HEAD, D_FFN = 1024, 16, 4096
DH = D_MODEL // N_HEAD
MAX_REL = 128
B, T = 2, 2048
N_CORES = 8
QT = 512            # tokens per core (query block)
SLAB_W = 2432       # slab columns: max m0 (1920) + 512
DT = D_MODEL // 128  # 8 d-tiles
FT = D_FFN // 128    # 32 ffn tiles
TT = T // 128        # 16 token tiles
NCOL_CONSTS = 16

_STATE: dict = {}


# ---------------------------------------------------------------------------
# numpy fallback (exact reference math)
# ---------------------------------------------------------------------------

def _softplus10(b_raw):
    return np.logaddexp(0.0, 10.0 * np.asarray(b_raw, np.float64)) / 10.0


def _kernel_numpy(x, casual_mask, Wq, bq, Wk, bk, Wv, bv, Wo, bo, rel_emb,
                  g1, be1, a1, br1, g2, be2, a2, br2, W1, b1, W2, b2,
                  res_scale):
    def aln(x, gamma, beta, a, b_raw, eps=1e-5):
        mean = x.mean(axis=-1, keepdims=True)
        var = x.var(axis=-1, keepdims=True)
        z = var + eps
        b = _softplus10(b_raw).astype(np.float32)
        p = a[0] + a[1] * z + a[2] * z * z
        q = b[0] + b[1] * z + b[2] * z * z
        return (x - mean) * (p / q) * gamma + beta

    x = np.asarray(x, np.float32)
    scale = np.clip(np.float32(res_scale), 0.2, 1.0)
    h1 = aln(x, g1, be1, a1, br1)
    Q = (h1 @ Wq.T + bq).reshape(B, T, N_HEAD, DH)
    K = (h1 @ Wk.T + bk).reshape(B, T, N_HEAD, DH)
    V = (h1 @ Wv.T + bv).reshape(B, T, N_HEAD, DH)
    sc = np.float32(DH ** -0.5)
    mask = np.asarray(casual_mask, bool)
    rel = np.arange(T)[None, :] - np.arange(T)[:, None]
    buckets = np.clip(rel, -MAX_REL + 1, MAX_REL - 1) + (MAX_REL - 1)
    bias = np.asarray(rel_emb, np.float32)[buckets]  # [T,T,H]
    out = np.empty((B, T, D_MODEL), np.float32)
    for b_i in range(B):
        s = np.einsum('qhd,khd->hqk', Q[b_i], K[b_i]) * sc
        s = s + bias.transpose(2, 0, 1)
        s = np.where(mask[None], s, 0.0)
        w = np.maximum(s, 0.0) + 1e-6
        w = np.where(mask[None], w, 0.0)
        w = w / (w.sum(axis=-1, keepdims=True) + 1e-6)
        ctx = np.einsum('hqk,khd->qhd', w, V[b_i]).reshape(T, D_MODEL)
        out[b_i] = ctx
    attn = out.reshape(B * T, D_MODEL) @ Wo.T + bo
    x1 = x + scale * attn.reshape(B, T, D_MODEL)
    h2 = aln(x1, g2, be2, a2, br2)
    ffn = np.maximum(h2.reshape(B * T, D_MODEL) @ W1.T + b1, 0.0) @ W2.T + b2
    return (x1 + scale * ffn.reshape(B, T, D_MODEL)).astype(np.float32)


# ---------------------------------------------------------------------------
# device program
# ---------------------------------------------------------------------------

def _ensure_hooks():
    """Register the NTFF profile hook (used by test.py tracing only)."""
    try:
        import antenv
        if "antenv.axon_hooks" not in sys.modules:
            m = types.ModuleType("antenv.axon_hooks")
            holder = {"hook": None}
            m.set_axon_ntff_profile_hook = lambda h: holder.__setitem__("hook", h)
            m.get_axon_ntff_profile_hook = lambda: holder["hook"]
            sys.modules["antenv.axon_hooks"] = m
            antenv.axon_hooks = m
        from antenv.axon_hooks import (
            get_axon_ntff_profile_hook,
            set_axon_ntff_profile_hook,
        )
        if get_axon_ntff_profile_hook() is None:
            from trn_agent_boot.trn_boot import _ntff_profile_via_ctypes
            set_axon_ntff_profile_hook(
                _ntff_profile_via_ctypes('/opt/axon/libaxon_pjrt.so'))
    except Exception:
        pass


def _build_nc():
    import concourse.bass as bass
    import concourse.mybir as mybir
    import concourse.tile as tile
    from concourse import bacc

    f32 = mybir.dt.float32
    bf16 = mybir.dt.bfloat16

    nc = bacc.Bacc("TRN2", target_bir_lowering=False, debug=False)

    xT = nc.dram_tensor("xT", [D_MODEL, T], bf16, kind="ExternalInput")
    xqT = nc.dram_tensor("xqT", [D_MODEL, QT], bf16, kind="ExternalInput")
    wqT = nc.dram_tensor("wqT", [D_MODEL, D_MODEL], bf16, kind="ExternalInput")
    wkT = nc.dram_tensor("wkT", [D_MODEL, D_MODEL], bf16, kind="ExternalInput")
    wvT = nc.dram_tensor("wvT", [D_MODEL, D_MODEL], bf16, kind="ExternalInput")
    woT = nc.dram_tensor("woT", [D_MODEL, D_MODEL], bf16, kind="ExternalInput")
    w1T = nc.dram_tensor("w1T", [D_MODEL, D_FFN], bf16, kind="ExternalInput")
    w2T = nc.dram_tensor("w2T", [D_FFN, D_MODEL], bf16, kind="ExternalInput")
    slab = nc.dram_tensor("slab", [N_HEAD, 128, SLAB_W], bf16,
                          kind="ExternalInput")
    meps = nc.dram_tensor("meps", [128, SLAB_W], bf16, kind="ExternalInput")
    bqc = nc.dram_tensor("bqc", [128, DT], f32, kind="ExternalInput")
    bkc = nc.dram_tensor("bkc", [128, DT], f32, kind="ExternalInput")
    boc = nc.dram_tensor("boc", [128, DT], f32, kind="ExternalInput")
    b1c = nc.dram_tensor("b1c", [128, FT], f32, kind="ExternalInput")
    b2c = nc.dram_tensor("b2c", [128, DT], f32, kind="ExternalInput")
    consts = nc.dram_tensor("consts", [128, NCOL_CONSTS], f32,
                            kind="ExternalInput")
    out_d = nc.dram_tensor("out", [D_MODEL, QT], f32, kind="ExternalOutput")

    ts = bass.ts

    from contextlib import ExitStack

    with tile.TileContext(nc) as tc:
        with ExitStack() as es:
            def mkpool(stack, name, bufs, **kw):
                return stack.enter_context(
                    tc.tile_pool(name=name, bufs=bufs, **kw))

            # LEFT side: long-lived pools (never closed mid-program)
            constp = mkpool(es, "const1", 1, side="left")
            xqp = mkpool(es, "xq", 1, side="left")
            bcp = mkpool(es, "bcast", 2, side="left")
            ptp = mkpool(es, "ptile", 3, side="left")
            smallp = mkpool(es, "small", 2, side="left")
            outp = mkpool(es, "outp", 2, side="left")
            psp = mkpool(es, "psum", 4, space=bass.MemorySpace.PSUM)
            psctxp = mkpool(es, "psum_ctx", 2, space=bass.MemorySpace.PSUM)

            AF = mybir.ActivationFunctionType
            ALU = mybir.AluOpType

            # ---- constants ----
            cst = constp.tile([128, NCOL_CONSTS], f32, name="cst")
            nc.sync.dma_start(cst[:], consts.ap())
            ones_bf = constp.tile([128, 128], bf16, name="ones_bf")
            nc.vector.memset(ones_bf[:], 1.0)
            mepst = constp.tile([128, SLAB_W], bf16, name="mepst")
            nc.sync.dma_start(mepst[:], meps.ap())
            bq_sb = constp.tile([128, DT], f32, name="bq_sb")
            nc.sync.dma_start(bq_sb[:], bqc.ap())
            bk_sb = constp.tile([128, DT], f32, name="bk_sb")
            nc.sync.dma_start(bk_sb[:], bkc.ap())
            bo_sb = constp.tile([128, DT], f32, name="bo_sb")
            nc.sync.dma_start(bo_sb[:], boc.ap())
            b1_sb = constp.tile([128, FT], f32, name="b1_sb")
            nc.sync.dma_start(b1_sb[:], b1c.ap())
            b2_sb = constp.tile([128, DT], f32, name="b2_sb")
            nc.sync.dma_start(b2_sb[:], b2c.ap())

            SCALE = cst[:, 0:1]
            EPS6 = cst[0:1, 13:14]

            def ln_stats_apply(tp, src_tiles, ntok, pcol, qcol, dst_tiles):
                """Feature-major algebraic LN (stats replicated across
                partitions via all-ones stationary matmul)."""
                nchunk = ntok // 512
                for ch in range(nchunk):
                    sl = ts(ch, 512)
                    psA = psp.tile([128, 512], f32, tag="mm", name="psA")
                    for d in range(DT):
                        nc.tensor.matmul(psA[:], ones_bf[:],
                                         src_tiles[d][:, sl],
                                         start=(d == 0), stop=(d == DT - 1))
                    psB = psp.tile([128, 512], f32, tag="mm", name="psB")
                    for d in range(DT):
                        xsq = tp.tile([128, 512], bf16, tag="xsq",
                                      name="xsq")
                        nc.vector.tensor_mul(xsq[:], src_tiles[d][:, sl],
                                             src_tiles[d][:, sl])
                        nc.tensor.matmul(psB[:], ones_bf[:], xsq[:],
                                         start=(d == 0), stop=(d == DT - 1))
                    mean = bcp.tile([128, 512], f32, tag="mean", name="mean")
                    nc.scalar.activation(mean[:], psA[:], AF.Copy,
                                         scale=1.0 / D_MODEL)
                    m2 = tp.tile([128, 512], f32, tag="m2", name="m2")
                    nc.vector.tensor_mul(m2[:], mean[:], mean[:])
                    var = tp.tile([128, 512], f32, tag="var", name="var")
                    nc.vector.scalar_tensor_tensor(
                        var[:], psB[:], 1.0 / D_MODEL, m2[:],
                        op0=ALU.mult, op1=ALU.subtract)
                    pt = tp.tile([128, 512], f32, tag="pt", name="pt")
                    nc.vector.tensor_scalar(pt[:], var[:],
                                            cst[:, pcol + 2:pcol + 3],
                                            cst[:, pcol + 1:pcol + 2],
                                            op0=ALU.mult, op1=ALU.add)
                    nc.vector.tensor_mul(pt[:], pt[:], var[:])
                    nc.vector.tensor_scalar_add(pt[:], pt[:],
                                                cst[:, pcol:pcol + 1])
                    qt = tp.tile([128, 512], f32, tag="qt", name="qt")
                    nc.vector.tensor_scalar(qt[:], var[:],
                                            cst[:, qcol + 2:qcol + 3],
                                            cst[:, qcol + 1:qcol + 2],
                                            op0=ALU.mult, op1=ALU.add)
                    nc.vector.tensor_mul(qt[:], qt[:], var[:])
                    nc.vector.tensor_scalar_add(qt[:], qt[:],
                                                cst[:, qcol:qcol + 1])
                    nc.vector.reciprocal(qt[:], qt[:])
                    pq = bcp.tile([128, 512], f32, tag="pq", name="pq")
                    nc.vector.tensor_mul(pq[:], pt[:], qt[:])
                    for d in range(DT):
                        ctr = tp.tile([128, 512], f32, tag="ctr",
                                      name="ctr")
                        nc.vector.tensor_sub(ctr[:], src_tiles[d][:, sl],
                                             mean[:])
                        nc.vector.tensor_mul(dst_tiles[d][:, sl], ctr[:],
                                             pq[:])

            xq = [xqp.tile([128, QT], bf16, tag=f"xq{d}", name=f"xq{d}")
                  for d in range(DT)]
            for d in range(DT):
                nc.sync.dma_start(xq[d][:], xqT.ap()[ts(d, 128), :])

            # RIGHT side stack (LIFO): attention data at the bottom, then
            # h1, then transient x / weight / tmp pools on top.
            esAtt = ExitStack()
            qTp = mkpool(esAtt, "qT", 1, side="right")
            kTp = mkpool(esAtt, "kT", 1, side="right")
            vtokp = mkpool(esAtt, "vtok", 1, side="right")
            esH = ExitStack()
            h1p = mkpool(esH, "h1", 1, side="right")
            h1qp = mkpool(esH, "h1q", 1, side="right")
            esA = ExitStack()
            xinp = mkpool(esA, "xin", 1, side="right")
            tmpLN1 = mkpool(esA, "tmpLN1", 2, side="right")

            # ---- load x, LN1 over full batch and own block ----
            xt = [xinp.tile([128, T], bf16, tag=f"x{d}", name=f"x{d}")
                  for d in range(DT)]
            for d in range(DT):
                nc.sync.dma_start(xt[d][:], xT.ap()[ts(d, 128), :])

            h1 = [h1p.tile([128, T], bf16, tag=f"h1{d}", name=f"h1{d}")
                  for d in range(DT)]
            ln_stats_apply(tmpLN1, xt, T, 1, 4, h1)
            h1q = [h1qp.tile([128, QT], bf16, tag=f"h1q{d}", name=f"h1q{d}")
                   for d in range(DT)]
            ln_stats_apply(tmpLN1, xq, QT, 1, 4, h1q)
            esA.close()

            # ---- QKV projections ----
            esW = ExitStack()
            wqp = mkpool(esW, "wq", 1, side="right")
            wkp = mkpool(esW, "wk", 1, side="right")
            wvp = mkpool(esW, "wv", 1, side="right")

            wq_sb = [wqp.tile([128, D_MODEL], bf16, tag=f"wq{i}",
                              name=f"wq{i}") for i in range(DT)]
            for i in range(DT):
                nc.sync.dma_start(wq_sb[i][:], wqT.ap()[ts(i, 128), :])
            wk_sb = [wkp.tile([128, D_MODEL], bf16, tag=f"wk{i}",
                              name=f"wk{i}") for i in range(DT)]
            for i in range(DT):
                nc.sync.dma_start(wk_sb[i][:], wkT.ap()[ts(i, 128), :])
            wv_sb = [wvp.tile([128, D_MODEL], bf16, tag=f"wv{i}",
                              name=f"wv{i}") for i in range(DT)]
            for i in range(DT):
                nc.sync.dma_start(wv_sb[i][:], wvT.ap()[ts(i, 128), :])

            qT_sb = [qTp.tile([128, QT], bf16, tag=f"q{o}", name=f"q{o}")
                     for o in range(DT)]
            for o in range(DT):
                ps = psp.tile([128, QT], f32, tag="mm", name="ps_q")
                for i in range(DT):
                    nc.tensor.matmul(ps[:], wq_sb[i][:, ts(o, 128)],
                                     h1q[i][:], start=(i == 0),
                                     stop=(i == DT - 1))
                nc.scalar.activation(qT_sb[o][:], ps[:], AF.Identity,
                                     bias=bq_sb[:, o:o + 1])

            kT_sb = [kTp.tile([128, T], bf16, tag=f"k{o}", name=f"k{o}")
                     for o in range(DT)]
            for o in range(DT):
                for ch in range(T // 512):
                    ps = psp.tile([128, 512], f32, tag="mm", name="ps_k")
                    for i in range(DT):
                        nc.tensor.matmul(ps[:], wk_sb[i][:, ts(o, 128)],
                                         h1[i][:, ts(ch, 512)],
                                         start=(i == 0), stop=(i == DT - 1))
                    nc.scalar.activation(kT_sb[o][:, ts(ch, 512)], ps[:],
                                         AF.Identity, bias=bk_sb[:, o:o + 1])

            # V token-major [k, 16, 65]; col 64 of each head = 1.0 (rowsums)
            vtok = [vtokp.tile([128, N_HEAD, DH + 1], bf16, tag=f"v{t}",
                               name=f"v{t}") for t in range(TT)]
            for t in range(TT):
                nc.vector.memset(vtok[t][:, :, DH:DH + 1], 1.0)
                for ch in range(2):
                    ps = psp.tile([128, 512], f32, tag="mm", name="ps_v")
                    for i in range(DT):
                        nc.tensor.matmul(ps[:], h1[i][:, ts(t, 128)],
                                         wv_sb[i][:, ts(ch, 512)],
                                         start=(i == 0), stop=(i == DT - 1))
                    nc.scalar.activation(
                        vtok[t][:, ch * 8:(ch + 1) * 8, 0:DH],
                        ps.rearrange("p (h x) -> p h x", h=8),
                        AF.Copy)
            esW.close()
            esH.close()

            # ---- attention, per head ----
            esS = ExitStack()
            slabp = mkpool(esS, "slabp", 2, side="right")
            saddp = mkpool(esS, "sadd", 3, side="right")
            ctxTp = mkpool(es, "ctxT", 1, side="left")
            ctxT_sb = [ctxTp.tile([128, QT], bf16, tag=f"ctx{d}",
                                  name=f"ctx{d}") for d in range(DT)]
            for h in range(N_HEAD):
                dt_i, poff = h // 2, (h % 2) * DH
                slab_sb = slabp.tile([128, SLAB_W], bf16, tag="slab",
                                     name="slab_sb")
                nc.sync.dma_start(slab_sb[:], slab.ap()[h, :, :])
                ctx_ps = psctxp.tile([DH + 1, QT], f32, tag="cps",
                                     name="ctx_ps")
                for kj in range(TT):
                    s_ps = psp.tile([128, QT], f32, tag="mm", name="s_ps")
                    nc.tensor.matmul(
                        s_ps[:],
                        kT_sb[dt_i][poff:poff + DH, ts(kj, 128)],
                        qT_sb[dt_i][poff:poff + DH, :],
                        start=True, stop=True)
                    m0 = 1920 - 128 * kj
                    t1 = saddp.tile([128, QT], f32, tag="sadd", name="t1")
                    nc.vector.tensor_add(t1[:], s_ps[:],
                                         slab_sb[:, m0:m0 + QT])
                    pt_sb = ptp.tile([128, QT], bf16, tag="ptile",
                                     name="pt_sb")
                    nc.vector.scalar_tensor_tensor(
                        pt_sb[:], t1[:], 0.0, mepst[:, m0:m0 + QT],
                        op0=ALU.max, op1=ALU.add)
                    nc.tensor.matmul(ctx_ps[:], vtok[kj][:, h, :], pt_sb[:],
                                     start=(kj == 0), stop=(kj == TT - 1))
                rden = smallp.tile([1, QT], f32, tag="rden", name="rden")
                nc.vector.tensor_scalar_add(rden[:], ctx_ps[DH:DH + 1, :],
                                            EPS6)
                nc.vector.reciprocal(rden[:], rden[:])
                rbc = smallp.tile([DH, QT], f32, tag="rbc", name="rbc")
                nc.gpsimd.partition_broadcast(rbc[:], rden[:])
                nc.vector.tensor_mul(ctxT_sb[dt_i][poff:poff + DH, :],
                                     ctx_ps[0:DH, :], rbc[:])
            esS.close()
            esAtt.close()

            # ---- out-proj + residual ----
            esWo = ExitStack()
            wop = mkpool(esWo, "wo", 1, side="right")
            abp = mkpool(esWo, "abp", 2, side="right")
            x1Tp = mkpool(es, "x1T", 1, side="left")
            wo_sb = [wop.tile([128, D_MODEL], bf16, tag=f"wo{i}",
                              name=f"wo{i}") for i in range(DT)]
            for i in range(DT):
                nc.sync.dma_start(wo_sb[i][:], woT.ap()[ts(i, 128), :])
            x1T = [x1Tp.tile([128, QT], bf16, tag=f"x1{d}", name=f"x1{d}")
                   for d in range(DT)]
            for o in range(DT):
                ps = psp.tile([128, QT], f32, tag="mm", name="ps_o")
                for c in range(DT):
                    nc.tensor.matmul(ps[:], wo_sb[c][:, ts(o, 128)],
                                     ctxT_sb[c][:], start=(c == 0),
                                     stop=(c == DT - 1))
                ab = abp.tile([128, QT], bf16, tag="ab", name="ab")
                nc.scalar.activation(ab[:], ps[:], AF.Identity,
                                     bias=bo_sb[:, o:o + 1])
                nc.vector.scalar_tensor_tensor(
                    x1T[o][:], ab[:], SCALE, xq[o][:],
                    op0=ALU.mult, op1=ALU.add)
            esWo.close()

            # ---- LN2 + FFN + final residual ----
            esL2 = ExitStack()
            tmpLN2 = mkpool(esL2, "tmpLN2", 2, side="right")
            h2Tp = mkpool(es, "h2T", 1, side="left")
            h2 = [h2Tp.tile([128, QT], bf16, tag=f"h2{d}", name=f"h2{d}")
                  for d in range(DT)]
            ln_stats_apply(tmpLN2, x1T, QT, 7, 10, h2)
            esL2.close()

            esF = ExitStack()
            midTp = mkpool(es, "midT", 1, side="left")
            w1p = mkpool(esF, "w1", 2, side="right")
            w2p = mkpool(esF, "w2", 2, side="right")
            fbp = mkpool(esF, "fbp", 2, side="right")
            mid = [midTp.tile([128, QT], bf16, tag=f"mid{f}", name=f"mid{f}")
                   for f in range(FT)]
            for fo in range(FT):
                w1_sb = w1p.tile([128, DT, 128], bf16, tag="w1", name="w1_sb")
                nc.sync.dma_start(
                    w1_sb[:],
                    w1T.ap()[:, ts(fo, 128)].rearrange(
                        "(a p) n -> p a n", p=128))
                ps = psp.tile([128, QT], f32, tag="mm", name="ps_f1")
                for i in range(DT):
                    nc.tensor.matmul(ps[:], w1_sb[:, i, :],
                                     h2[i][:], start=(i == 0),
                                     stop=(i == DT - 1))
                nc.scalar.activation(mid[fo][:], ps[:], AF.Relu,
                                     bias=b1_sb[:, fo:fo + 1])

            for o in range(DT):
                w2_sb = w2p.tile([128, FT, 128], bf16, tag="w2", name="w2_sb")
                nc.sync.dma_start(
                    w2_sb[:],
                    w2T.ap()[:, ts(o, 128)].rearrange(
                        "(a p) n -> p a n", p=128))
                ps = psp.tile([128, QT], f32, tag="mm", name="ps_f2")
                for fi in range(FT):
                    nc.tensor.matmul(ps[:], w2_sb[:, fi, :],
                                     mid[fi][:], start=(fi == 0),
                                     stop=(fi == FT - 1))
                fb = fbp.tile([128, QT], bf16, tag="fb", name="fb")
                nc.scalar.activation(fb[:], ps[:], AF.Identity,
                                     bias=b2_sb[:, o:o + 1])
                of = outp.tile([128, QT], f32, tag="of", name="of")
                nc.vector.scalar_tensor_tensor(
                    of[:], fb[:], SCALE, x1T[o][:],
                    op0=ALU.mult, op1=ALU.add)
                nc.sync.dma_start(out_d.ap()[ts(o, 128), :], of[:])
            esF.close()

    nc.compile()
    return nc


# ---------------------------------------------------------------------------
# host-side prep
# ---------------------------------------------------------------------------

def _prep_in_maps(inputs):
    import ml_dtypes
    bf = ml_dtypes.bfloat16

    x = np.asarray(inputs["x"], np.float32)
    Wq = np.asarray(inputs["Wq"], np.float32)
    Wk = np.asarray(inputs["Wk"], np.float32)
    Wv = np.asarray(inputs["Wv"], np.float32)
    Wo = np.asarray(inputs["Wo"], np.float32)
    W1 = np.asarray(inputs["W1"], np.float32)
    W2 = np.asarray(inputs["W2"], np.float32)
    bq = np.asarray(inputs["bq"], np.float32)
    bk = np.asarray(inputs["bk"], np.float32)
    bv = np.asarray(inputs["bv"], np.float32)
    bo = np.asarray(inputs["bo"], np.float32)
    b1 = np.asarray(inputs["b1"], np.float32)
    b2 = np.asarray(inputs["b2"], np.float32)
    g1 = np.asarray(inputs["g1"], np.float32)
    be1 = np.asarray(inputs["be1"], np.float32)
    g2 = np.asarray(inputs["g2"], np.float32)
    be2 = np.asarray(inputs["be2"], np.float32)
    a1 = np.asarray(inputs["a1"], np.float64)
    br1 = np.asarray(inputs["br1"], np.float64)
    a2 = np.asarray(inputs["a2"], np.float64)
    br2 = np.asarray(inputs["br2"], np.float64)
    rel_emb = np.asarray(inputs["rel_emb"], np.float32)
    res_scale = np.float32(np.clip(np.asarray(inputs["res_scale"],
                                              np.float32), 0.2, 1.0))
    mask = np.asarray(inputs["casual_mask"], bool)

    # device path requires the canonical causal mask and zero V-bias
    bvp = Wv @ be1 + bv
    if not np.array_equal(mask, np.tril(np.ones((T, T), bool))):
        raise ValueError("non-causal mask unsupported on device")
    if np.any(bvp != 0):
        raise ValueError("nonzero folded V bias unsupported on device")

    sc = np.float32(DH ** -0.5)
    wqT = np.ascontiguousarray(((Wq * g1[None, :]).T * sc).astype(bf))
    bqp = ((Wq @ be1 + bq) * sc).astype(np.float32)
    wkT = np.ascontiguousarray((Wk * g1[None, :]).T.astype(bf))
    bkp = (Wk @ be1 + bk).astype(np.float32)
    wvT = np.ascontiguousarray((Wv * g1[None, :]).T.astype(bf))
    woT = np.ascontiguousarray(Wo.T.astype(bf))
    bop = bo
    w1T = np.ascontiguousarray((W1 * g2[None, :]).T.astype(bf))
    b1p = (W1 @ be2 + b1).astype(np.float32)
    w2T = np.ascontiguousarray(W2.T.astype(bf))
    b2p = b2

    # shifted LN polynomials (fold the +eps into the coefficients)
    eps = 1e-5

    def shift(c0, c1, c2):
        return (np.float32(c0 + c1 * eps + c2 * eps * eps),
                np.float32(c1 + 2 * c2 * eps), np.float32(c2))

    p1 = shift(*a1)
    sb1 = _softplus10(br1)
    q1 = shift(*sb1)
    p2 = shift(*a2)
    sb2 = _softplus10(br2)
    q2 = shift(*sb2)

    consts = np.zeros((128, NCOL_CONSTS), np.float32)
    consts[:, 0] = res_scale
    consts[:, 1:4] = p1
    consts[:, 4:7] = q1
    consts[:, 7:10] = p2
    consts[:, 10:13] = q2
    consts[:, 13] = 1e-6

    # Toeplitz bias+mask slabs, one per q-block
    # band_full[h, i], i = (k - q) + 2047: rel bias if k<=q else -1e9
    i_ax = np.arange(4096)
    kq = i_ax - 2047
    buck = np.clip(kq, -MAX_REL + 1, MAX_REL - 1) + (MAX_REL - 1)
    band = rel_emb[buck, :].T.astype(np.float32)  # [H, 4096]
    band[:, kq > 0] = -1e9
    pp = np.arange(128)[:, None]
    mm = np.arange(SLAB_W)[None, :]
    slabs, mepss = {}, {}
    for qb in range(4):
        q0 = qb * QT
        idx = pp - mm + 3967 - q0
        slabs[qb] = np.ascontiguousarray(band[:, idx].astype(bf))
        mepss[qb] = np.ascontiguousarray(
            np.where(idx <= 2047, np.float32(1e-6), np.float32(0.0)
                     ).astype(bf))

    def colpack(v):
        return np.ascontiguousarray(v.reshape(-1, 128).T.astype(np.float32))

    bq_c, bk_c, bo_c = colpack(bqp), colpack(bkp), colpack(bop)
    b1_c, b2_c = colpack(b1p), colpack(b2p)

    xT = {b_i: np.ascontiguousarray(x[b_i].T.astype(bf)) for b_i in range(B)}

    in_maps = []
    for c in range(N_CORES):
        b_i, qb = c // 4, c % 4
        in_maps.append({
            "xT": xT[b_i],
            "xqT": np.ascontiguousarray(xT[b_i][:, qb * QT:(qb + 1) * QT]),
            "wqT": wqT, "wkT": wkT, "wvT": wvT, "woT": woT,
            "w1T": w1T, "w2T": w2T,
            "slab": slabs[qb], "meps": mepss[qb],
            "bqc": bq_c, "bkc": bk_c, "boc": bo_c,
            "b1c": b1_c, "b2c": b2_c,
            "consts": consts,
        })
    return in_maps


def _assemble(results):
    out = np.empty((B, T, D_MODEL), np.float32)
    for c in range(N_CORES):
        b_i, qb = c // 4, c % 4
        out[b_i, qb * QT:(qb + 1) * QT, :] = results[c]["out"].T
    return out


def _fix_first_rows(out, inputs, nrows=32):
    """Recompute the first `nrows` query rows of each batch exactly.

    Early rows have tiny attention mass (sum of relu'd scores ~ q), so the
    relu sign of near-zero bf16 scores can materially change their weights.
    Causality means row q depends only on tokens <= q, so an exact fp32
    recompute of the first rows costs microseconds.
    """
    sub = {k: v for k, v in inputs.items()}
    sub["x"] = np.asarray(inputs["x"], np.float32)[:, :nrows, :]
    sub["casual_mask"] = np.asarray(inputs["casual_mask"],
                                    bool)[:nrows, :nrows]
    global T
    t_save = T
    try:
        T = nrows
        out[:, :nrows, :] = _kernel_numpy(**sub)
    finally:
        T = t_save
    return out


def _get_nc():
    if "nc" not in _STATE:
        _ensure_hooks()
        _STATE["nc"] = _build_nc()
    return _STATE["nc"]


def kernel(**inputs):
    inputs = {k: np.asarray(v) for k, v in inputs.items()}
    try:
        from concourse.bass_utils import run_bass_kernel_spmd
        nc = _get_nc()
        in_maps = _prep_in_maps(inputs)
        res = run_bass_kernel_spmd(nc, in_maps, core_ids=list(range(N_CORES)))
        out = _assemble(res.results)
        if not np.all(np.isfinite(out)):
            raise ValueError("non-finite device output")
        return _fix_first_rows(out, inputs)
    except Exception as e:
        import traceback
        traceback.print_exc()
        print(f"device path failed ({e!r}); using numpy fallback",
              file=sys.stderr)
        return _kernel_numpy(**inputs)


# revision 14
# speedup vs baseline: 1.0824x; 1.0824x over previous
"""AlgebraicTransformerBlock on 8 Trainium2 NeuronCores (Bass/Tile).

Sharding: SPMD, one program on all 8 cores. Core c owns batch b = c//4 and
the 512-token query block qb = c%4 of that batch. No collectives: each core
redundantly computes LN1 + K/V projections for its batch's full 2048 tokens
(needed for causal attention), then attention / out-proj / LN2 / FFN for its
own 512 queries, and returns a [1024, 512] feature-major output slice that
the host transposes and concatenates.

Device layout is feature-major ([d_model, tokens]) throughout:
 - LN mean / E[x^2] are computed with an all-ones stationary matmul, which
   leaves the per-token stats replicated across all 128 partitions — exactly
   the broadcast form the normalize step needs. No transposes anywhere.
 - LN gamma/beta and the attention 1/sqrt(dh) are folded into the weights
   host-side, so the device only computes (x - mean) * p(var)/q(var).
 - The causal mask and the Toeplitz relative bias are folded into one
   per-core [16, 128, 2432] bf16 "slab" (bias value where visible, -1e9
   where masked): score tile (head h, k-tile kj) adds slab[h][:, 1920-128*kj
   :][:, :512]. The reference's `w = relu(s)+1e-6 (masked)` becomes one
   fused DVE op: P = relu(S + slab) + meps, with meps the 1e-6*mask slab.
 - V is produced token-major [k, 16, 65] with column 64 of each head set to
   1.0, so the context matmul P^T-accumulation also yields the row-sums
   needed for normalization; the per-query reciprocal is applied per head
   (partition-broadcast), exactly preserving the reference epsilon algebra.

The host-side `kernel()` folds/casts weights (bf16), builds the slabs, runs
the program via run_bass_kernel_spmd, and reassembles the full [2, 2048,
1024] fp32 output. A numpy fallback reproduces the exact math if the device
path is unavailable.
"""

import sys
import types

import numpy as np

D_MODEL, N_HEAD, D_FFN = 1024, 16, 4096
DH = D_MODEL // N_HEAD
MAX_REL = 128
B, T = 2, 2048
N_CORES = 8
QT = 512            # tokens per core (query block)
SLAB_W = 2432       # slab columns: max m0 (1920) + 512
DT = D_MODEL // 128  # 8 d-tiles
FT = D_FFN // 128    # 32 ffn tiles
TT = T // 128        # 16 token tiles
NCOL_CONSTS = 16

_STATE: dict = {}


# ---------------------------------------------------------------------------
# numpy fallback (exact reference math)
# ---------------------------------------------------------------------------

def _softplus10(b_raw):
    return np.logaddexp(0.0, 10.0 * np.asarray(b_raw, np.float64)) / 10.0


def _kernel_numpy(x, casual_mask, Wq, bq, Wk, bk, Wv, bv, Wo, bo, rel_emb,
                  g1, be1, a1, br1, g2, be2, a2, br2, W1, b1, W2, b2,
                  res_scale):
    def aln(x, gamma, beta, a, b_raw, eps=1e-5):
        mean = x.mean(axis=-1, keepdims=True)
        var = x.var(axis=-1, keepdims=True)
        z = var + eps
        b = _softplus10(b_raw).astype(np.float32)
        p = a[0] + a[1] * z + a[2] * z * z
        q = b[0] + b[1] * z + b[2] * z * z
        return (x - mean) * (p / q) * gamma + beta

    x = np.asarray(x, np.float32)
    scale = np.clip(np.float32(res_scale), 0.2, 1.0)
    h1 = aln(x, g1, be1, a1, br1)
    Q = (h1 @ Wq.T + bq).reshape(B, T, N_HEAD, DH)
    K = (h1 @ Wk.T + bk).reshape(B, T, N_HEAD, DH)
    V = (h1 @ Wv.T + bv).reshape(B, T, N_HEAD, DH)
    sc = np.float32(DH ** -0.5)
    mask = np.asarray(casual_mask, bool)
    rel = np.arange(T)[None, :] - np.arange(T)[:, None]
    buckets = np.clip(rel, -MAX_REL + 1, MAX_REL - 1) + (MAX_REL - 1)
    bias = np.asarray(rel_emb, np.float32)[buckets]  # [T,T,H]
    out = np.empty((B, T, D_MODEL), np.float32)
    for b_i in range(B):
        s = np.einsum('qhd,khd->hqk', Q[b_i], K[b_i]) * sc
        s = s + bias.transpose(2, 0, 1)
        s = np.where(mask[None], s, 0.0)
        w = np.maximum(s, 0.0) + 1e-6
        w = np.where(mask[None], w, 0.0)
        w = w / (w.sum(axis=-1, keepdims=True) + 1e-6)
        ctx = np.einsum('hqk,khd->qhd', w, V[b_i]).reshape(T, D_MODEL)
        out[b_i] = ctx
    attn = out.reshape(B * T, D_MODEL) @ Wo.T + bo
    x1 = x + scale * attn.reshape(B, T, D_MODEL)
    h2 = aln(x1, g2, be2, a2, br2)
    ffn = np.maximum(h2.reshape(B * T, D_MODEL) @ W1.T + b1, 0.0) @ W2.T + b2
    return (x1 + scale * ffn.reshape(B, T, D_MODEL)).astype(np.float32)


# ---------------------------------------------------------------------------
# device program
# ---------------------------------------------------------------------------

def _ensure_hooks():
    """Register the NTFF profile hook (used by test.py tracing only)."""
    try:
        import antenv
        if "antenv.axon_hooks" not in sys.modules:
            m = types.ModuleType("antenv.axon_hooks")
            holder = {"hook": None}
            m.set_axon_ntff_profile_hook = lambda h: holder.__setitem__("hook", h)
            m.get_axon_ntff_profile_hook = lambda: holder["hook"]
            sys.modules["antenv.axon_hooks"] = m
            antenv.axon_hooks = m
        from antenv.axon_hooks import (
            get_axon_ntff_profile_hook,
            set_axon_ntff_profile_hook,
        )
        if get_axon_ntff_profile_hook() is None:
            from trn_agent_boot.trn_boot import _ntff_profile_via_ctypes
            set_axon_ntff_profile_hook(
                _ntff_profile_via_ctypes('/opt/axon/libaxon_pjrt.so'))
    except Exception:
        pass


def _build_nc():
    import concourse.bass as bass
    import concourse.mybir as mybir
    import concourse.tile as tile
    from concourse import bacc

    f32 = mybir.dt.float32
    bf16 = mybir.dt.bfloat16

    nc = bacc.Bacc("TRN2", target_bir_lowering=False, debug=False)

    xT = nc.dram_tensor("xT", [D_MODEL, T], bf16, kind="ExternalInput")
    xqT = nc.dram_tensor("xqT", [D_MODEL, QT], bf16, kind="ExternalInput")
    wqT = nc.dram_tensor("wqT", [D_MODEL, D_MODEL], bf16, kind="ExternalInput")
    wkT = nc.dram_tensor("wkT", [D_MODEL, D_MODEL], bf16, kind="ExternalInput")
    wvT = nc.dram_tensor("wvT", [D_MODEL, D_MODEL], bf16, kind="ExternalInput")
    woT = nc.dram_tensor("woT", [D_MODEL, D_MODEL], bf16, kind="ExternalInput")
    w1T = nc.dram_tensor("w1T", [D_MODEL, D_FFN], bf16, kind="ExternalInput")
    w2T = nc.dram_tensor("w2T", [D_FFN, D_MODEL], bf16, kind="ExternalInput")
    slab = nc.dram_tensor("slab", [N_HEAD, 128, SLAB_W], bf16,
                          kind="ExternalInput")
    meps = nc.dram_tensor("meps", [128, SLAB_W], bf16, kind="ExternalInput")
    bqc = nc.dram_tensor("bqc", [128, DT], f32, kind="ExternalInput")
    bkc = nc.dram_tensor("bkc", [128, DT], f32, kind="ExternalInput")
    boc = nc.dram_tensor("boc", [128, DT], f32, kind="ExternalInput")
    b1c = nc.dram_tensor("b1c", [128, FT], f32, kind="ExternalInput")
    b2c = nc.dram_tensor("b2c", [128, DT], f32, kind="ExternalInput")
    consts = nc.dram_tensor("consts", [128, NCOL_CONSTS], f32,
                            kind="ExternalInput")
    out_d = nc.dram_tensor("out", [D_MODEL, QT], f32, kind="ExternalOutput")

    ts = bass.ts

    from contextlib import ExitStack

    with tile.TileContext(nc) as tc:
        with ExitStack() as es:
            def mkpool(stack, name, bufs, **kw):
                return stack.enter_context(
                    tc.tile_pool(name=name, bufs=bufs, **kw))

            # LEFT side: long-lived pools (never closed mid-program)
            constp = mkpool(es, "const1", 1, side="left")
            xqp = mkpool(es, "xq", 1, side="left")
            bcp = mkpool(es, "bcast", 2, side="left")
            ptp = mkpool(es, "ptile", 3, side="left")
            smallp = mkpool(es, "small", 2, side="left")
            outp = mkpool(es, "outp", 2, side="left")
            psp = mkpool(es, "psum", 4, space=bass.MemorySpace.PSUM)
            psctxp = mkpool(es, "psum_ctx", 2, space=bass.MemorySpace.PSUM)

            AF = mybir.ActivationFunctionType
            ALU = mybir.AluOpType

            # ---- constants ----
            cst = constp.tile([128, NCOL_CONSTS], f32, name="cst")
            nc.sync.dma_start(cst[:], consts.ap())
            ones_bf = constp.tile([128, 128], bf16, name="ones_bf")
            nc.vector.memset(ones_bf[:], 1.0)
            mepst = constp.tile([128, SLAB_W], bf16, name="mepst")
            nc.sync.dma_start(mepst[:], meps.ap())
            bq_sb = constp.tile([128, DT], f32, name="bq_sb")
            nc.sync.dma_start(bq_sb[:], bqc.ap())
            bk_sb = constp.tile([128, DT], f32, name="bk_sb")
            nc.sync.dma_start(bk_sb[:], bkc.ap())
            bo_sb = constp.tile([128, DT], f32, name="bo_sb")
            nc.sync.dma_start(bo_sb[:], boc.ap())
            b1_sb = constp.tile([128, FT], f32, name="b1_sb")
            nc.sync.dma_start(b1_sb[:], b1c.ap())
            b2_sb = constp.tile([128, DT], f32, name="b2_sb")
            nc.sync.dma_start(b2_sb[:], b2c.ap())

            SCALE = cst[:, 0:1]
            EPS6 = cst[0:1, 13:14]

            def ln_stats_apply(tp, src_tiles, ntok, pcol, qcol, dst_tiles):
                """Feature-major algebraic LN (stats replicated across
                partitions via all-ones stationary matmul)."""
                nchunk = ntok // 512
                for ch in range(nchunk):
                    sl = ts(ch, 512)
                    psA = psp.tile([128, 512], f32, tag="mm", name="psA")
                    for d in range(DT):
                        nc.tensor.matmul(psA[:], ones_bf[:],
                                         src_tiles[d][:, sl],
                                         start=(d == 0), stop=(d == DT - 1))
                    psB = psp.tile([128, 512], f32, tag="mm", name="psB")
                    for d in range(DT):
                        xsq = tp.tile([128, 512], bf16, tag="xsq",
                                      name="xsq")
                        nc.vector.tensor_mul(xsq[:], src_tiles[d][:, sl],
                                             src_tiles[d][:, sl])
                        nc.tensor.matmul(psB[:], ones_bf[:], xsq[:],
                                         start=(d == 0), stop=(d == DT - 1))
                    mean = bcp.tile([128, 512], f32, tag="mean", name="mean")
                    nc.scalar.activation(mean[:], psA[:], AF.Copy,
                                         scale=1.0 / D_MODEL)
                    m2 = tp.tile([128, 512], f32, tag="m2", name="m2")
                    nc.vector.tensor_mul(m2[:], mean[:], mean[:])
                    var = tp.tile([128, 512], f32, tag="var", name="var")
                    nc.vector.scalar_tensor_tensor(
                        var[:], psB[:], 1.0 / D_MODEL, m2[:],
                        op0=ALU.mult, op1=ALU.subtract)
                    pt = tp.tile([128, 512], f32, tag="pt", name="pt")
                    nc.vector.tensor_scalar(pt[:], var[:],
                                            cst[:, pcol + 2:pcol + 3],
                                            cst[:, pcol + 1:pcol + 2],
                                            op0=ALU.mult, op1=ALU.add)
                    nc.vector.tensor_mul(pt[:], pt[:], var[:])
                    nc.vector.tensor_scalar_add(pt[:], pt[:],
                                                cst[:, pcol:pcol + 1])
                    qt = tp.tile([128, 512], f32, tag="qt", name="qt")
                    nc.vector.tensor_scalar(qt[:], var[:],
                                            cst[:, qcol + 2:qcol + 3],
                                            cst[:, qcol + 1:qcol + 2],
                                            op0=ALU.mult, op1=ALU.add)
                    nc.vector.tensor_mul(qt[:], qt[:], var[:])
                    nc.vector.tensor_scalar_add(qt[:], qt[:],
                                                cst[:, qcol:qcol + 1])
                    nc.vector.reciprocal(qt[:], qt[:])
                    pq = bcp.tile([128, 512], f32, tag="pq", name="pq")
                    nc.vector.tensor_mul(pq[:], pt[:], qt[:])
                    for d in range(DT):
                        ctr = tp.tile([128, 512], f32, tag="ctr",
                                      name="ctr")
                        nc.vector.tensor_sub(ctr[:], src_tiles[d][:, sl],
                                             mean[:])
                        nc.vector.tensor_mul(dst_tiles[d][:, sl], ctr[:],
                                             pq[:])

            xq = [xqp.tile([128, QT], bf16, tag=f"xq{d}", name=f"xq{d}")
                  for d in range(DT)]
            for d in range(DT):
                nc.sync.dma_start(xq[d][:], xqT.ap()[ts(d, 128), :])

            # RIGHT side stack (LIFO): attention data at the bottom, then
            # h1, then transient x / weight / tmp pools on top.
            esAtt = ExitStack()
            qTp = mkpool(esAtt, "qT", 1, side="right")
            kTp = mkpool(esAtt, "kT", 1, side="right")
            vtokp = mkpool(esAtt, "vtok", 1, side="right")
            esH = ExitStack()
            h1p = mkpool(esH, "h1", 1, side="right")
            h1qp = mkpool(esH, "h1q", 1, side="right")
            esA = ExitStack()
            xinp = mkpool(esA, "xin", 1, side="right")
            tmpLN1 = mkpool(esA, "tmpLN1", 2, side="right")

            # ---- load x, LN1 over full batch and own block ----
            xt = [xinp.tile([128, T], bf16, tag=f"x{d}", name=f"x{d}")
                  for d in range(DT)]
            for d in range(DT):
                nc.sync.dma_start(xt[d][:], xT.ap()[ts(d, 128), :])

            h1 = [h1p.tile([128, T], bf16, tag=f"h1{d}", name=f"h1{d}")
                  for d in range(DT)]
            ln_stats_apply(tmpLN1, xt, T, 1, 4, h1)
            h1q = [h1qp.tile([128, QT], bf16, tag=f"h1q{d}", name=f"h1q{d}")
                   for d in range(DT)]
            ln_stats_apply(tmpLN1, xq, QT, 1, 4, h1q)
            esA.close()

            # ---- QKV projections ----
            esW = ExitStack()
            wqp = mkpool(esW, "wq", 1, side="right")
            wkp = mkpool(esW, "wk", 1, side="right")
            wvp = mkpool(esW, "wv", 1, side="right")

            wq_sb = [wqp.tile([128, D_MODEL], bf16, tag=f"wq{i}",
                              name=f"wq{i}") for i in range(DT)]
            for i in range(DT):
                nc.sync.dma_start(wq_sb[i][:], wqT.ap()[ts(i, 128), :])
            wk_sb = [wkp.tile([128, D_MODEL], bf16, tag=f"wk{i}",
                              name=f"wk{i}") for i in range(DT)]
            for i in range(DT):
                nc.sync.dma_start(wk_sb[i][:], wkT.ap()[ts(i, 128), :])
            wv_sb = [wvp.tile([128, D_MODEL], bf16, tag=f"wv{i}",
                              name=f"wv{i}") for i in range(DT)]
            for i in range(DT):
                nc.sync.dma_start(wv_sb[i][:], wvT.ap()[ts(i, 128), :])

            qT_sb = [qTp.tile([128, QT], bf16, tag=f"q{o}", name=f"q{o}")
                     for o in range(DT)]
            for o in range(DT):
                ps = psp.tile([128, QT], f32, tag="mm", name="ps_q")
                for i in range(DT):
                    nc.tensor.matmul(ps[:], wq_sb[i][:, ts(o, 128)],
                                     h1q[i][:], start=(i == 0),
                                     stop=(i == DT - 1))
                nc.scalar.activation(qT_sb[o][:], ps[:], AF.Identity,
                                     bias=bq_sb[:, o:o + 1])

            kT_sb = [kTp.tile([128, T], bf16, tag=f"k{o}", name=f"k{o}")
                     for o in range(DT)]
            for o in range(DT):
                for ch in range(T // 512):
                    ps = psp.tile([128, 512], f32, tag="mm", name="ps_k")
                    for i in range(DT):
                        nc.tensor.matmul(ps[:], wk_sb[i][:, ts(o, 128)],
                                         h1[i][:, ts(ch, 512)],
                                         start=(i == 0), stop=(i == DT - 1))
                    nc.scalar.activation(kT_sb[o][:, ts(ch, 512)], ps[:],
                                         AF.Identity, bias=bk_sb[:, o:o + 1])

            # V token-major [k, 16, 65]; col 64 of each head = 1.0 (rowsums)
            vtok = [vtokp.tile([128, N_HEAD, DH + 1], bf16, tag=f"v{t}",
                               name=f"v{t}") for t in range(TT)]
            for t in range(TT):
                nc.vector.memset(vtok[t][:, :, DH:DH + 1], 1.0)
                for ch in range(2):
                    ps = psp.tile([128, 512], f32, tag="mm", name="ps_v")
                    for i in range(DT):
                        nc.tensor.matmul(ps[:], h1[i][:, ts(t, 128)],
                                         wv_sb[i][:, ts(ch, 512)],
                                         start=(i == 0), stop=(i == DT - 1))
                    nc.scalar.activation(
                        vtok[t][:, ch * 8:(ch + 1) * 8, 0:DH],
                        ps.rearrange("p (h x) -> p h x", h=8),
                        AF.Copy)
            esW.close()
            esH.close()

            # ---- attention, per head ----
            esS = ExitStack()
            slabp = mkpool(esS, "slabp", 2, side="right")
            saddp = mkpool(esS, "sadd", 3, side="right")
            ctxTp = mkpool(es, "ctxT", 1, side="left")
            ctxT_sb = [ctxTp.tile([128, QT], bf16, tag=f"ctx{d}",
                                  name=f"ctx{d}") for d in range(DT)]
            for h in range(N_HEAD):
                dt_i, poff = h // 2, (h % 2) * DH
                slab_sb = slabp.tile([128, SLAB_W], bf16, tag="slab",
                                     name="slab_sb")
                nc.sync.dma_start(slab_sb[:], slab.ap()[h, :, :])
                ctx_ps = psctxp.tile([DH + 1, QT], f32, tag="cps",
                                     name="ctx_ps")
                # epsilon prefix term 1e-6*sum_{k<=q} V_aug[k]: the meps
                # slab IS 1e-6*mask, so 16 mask-matmuls accumulate it
                # exactly; they depend on nothing but vtok, keeping PE busy
                # while the score pipeline fills.
                for kj in range(TT):
                    m0 = 1920 - 128 * kj
                    nc.tensor.matmul(ctx_ps[:], vtok[kj][:, h, :],
                                     mepst[:, m0:m0 + QT],
                                     start=(kj == 0), stop=False)
                for kj in range(TT):
                    s_ps = psp.tile([128, QT], f32, tag="mm", name="s_ps")
                    nc.tensor.matmul(
                        s_ps[:],
                        kT_sb[dt_i][poff:poff + DH, ts(kj, 128)],
                        qT_sb[dt_i][poff:poff + DH, :],
                        start=True, stop=True)
                    m0 = 1920 - 128 * kj
                    nc.vector.tensor_add(s_ps[:], s_ps[:],
                                         slab_sb[:, m0:m0 + QT])
                    pt_sb = ptp.tile([128, QT], bf16, tag="ptile",
                                     name="pt_sb")
                    nc.scalar.activation(pt_sb[:], s_ps[:], AF.Relu)
                    nc.tensor.matmul(ctx_ps[:], vtok[kj][:, h, :], pt_sb[:],
                                     start=False, stop=(kj == TT - 1))
                rden = smallp.tile([1, QT], f32, tag="rden", name="rden")
                nc.vector.tensor_scalar_add(rden[:], ctx_ps[DH:DH + 1, :],
                                            EPS6)
                nc.vector.reciprocal(rden[:], rden[:])
                rbc = smallp.tile([DH, QT], f32, tag="rbc", name="rbc")
                nc.gpsimd.partition_broadcast(rbc[:], rden[:])
                nc.vector.tensor_mul(ctxT_sb[dt_i][poff:poff + DH, :],
                                     ctx_ps[0:DH, :], rbc[:])
            esS.close()
            esAtt.close()

            # ---- out-proj + residual ----
            esWo = ExitStack()
            wop = mkpool(esWo, "wo", 1, side="right")
            abp = mkpool(esWo, "abp", 2, side="right")
            x1Tp = mkpool(es, "x1T", 1, side="left")
            wo_sb = [wop.tile([128, D_MODEL], bf16, tag=f"wo{i}",
                              name=f"wo{i}") for i in range(DT)]
            for i in range(DT):
                nc.sync.dma_start(wo_sb[i][:], woT.ap()[ts(i, 128), :])
            x1T = [x1Tp.tile([128, QT], bf16, tag=f"x1{d}", name=f"x1{d}")
                   for d in range(DT)]
            for o in range(DT):
                ps = psp.tile([128, QT], f32, tag="mm", name="ps_o")
                for c in range(DT):
                    nc.tensor.matmul(ps[:], wo_sb[c][:, ts(o, 128)],
                                     ctxT_sb[c][:], start=(c == 0),
                                     stop=(c == DT - 1))
                ab = abp.tile([128, QT], bf16, tag="ab", name="ab")
                nc.scalar.activation(ab[:], ps[:], AF.Identity,
                                     bias=bo_sb[:, o:o + 1])
                nc.vector.scalar_tensor_tensor(
                    x1T[o][:], ab[:], SCALE, xq[o][:],
                    op0=ALU.mult, op1=ALU.add)
            esWo.close()

            # ---- LN2 + FFN + final residual ----
            esL2 = ExitStack()
            tmpLN2 = mkpool(esL2, "tmpLN2", 2, side="right")
            h2Tp = mkpool(es, "h2T", 1, side="left")
            h2 = [h2Tp.tile([128, QT], bf16, tag=f"h2{d}", name=f"h2{d}")
                  for d in range(DT)]
            ln_stats_apply(tmpLN2, x1T, QT, 7, 10, h2)
            esL2.close()

            esF = ExitStack()
            midTp = mkpool(es, "midT", 1, side="left")
            w1p = mkpool(esF, "w1", 2, side="right")
            w2p = mkpool(esF, "w2", 2, side="right")
            fbp = mkpool(esF, "fbp", 2, side="right")
            mid = [midTp.tile([128, QT], bf16, tag=f"mid{f}", name=f"mid{f}")
                   for f in range(FT)]
            for fo in range(FT):
                w1_sb = w1p.tile([128, DT, 128], bf16, tag="w1", name="w1_sb")
                nc.sync.dma_start(
                    w1_sb[:],
                    w1T.ap()[:, ts(fo, 128)].rearrange(
                        "(a p) n -> p a n", p=128))
                ps = psp.tile([128, QT], f32, tag="mm", name="ps_f1")
                for i in range(DT):
                    nc.tensor.matmul(ps[:], w1_sb[:, i, :],
                                     h2[i][:], start=(i == 0),
                                     stop=(i == DT - 1))
                nc.scalar.activation(mid[fo][:], ps[:], AF.Relu,
                                     bias=b1_sb[:, fo:fo + 1])

            for o in range(DT):
                w2_sb = w2p.tile([128, FT, 128], bf16, tag="w2", name="w2_sb")
                nc.sync.dma_start(
                    w2_sb[:],
                    w2T.ap()[:, ts(o, 128)].rearrange(
                        "(a p) n -> p a n", p=128))
                ps = psp.tile([128, QT], f32, tag="mm", name="ps_f2")
                for fi in range(FT):
                    nc.tensor.matmul(ps[:], w2_sb[:, fi, :],
                                     mid[fi][:], start=(fi == 0),
                                     stop=(fi == FT - 1))
                fb = fbp.tile([128, QT], bf16, tag="fb", name="fb")
                nc.scalar.activation(fb[:], ps[:], AF.Identity,
                                     bias=b2_sb[:, o:o + 1])
                of = outp.tile([128, QT], f32, tag="of", name="of")
                nc.vector.scalar_tensor_tensor(
                    of[:], fb[:], SCALE, x1T[o][:],
                    op0=ALU.mult, op1=ALU.add)
                nc.sync.dma_start(out_d.ap()[ts(o, 128), :], of[:])
            esF.close()

    nc.compile()
    return nc


# ---------------------------------------------------------------------------
# host-side prep
# ---------------------------------------------------------------------------

def _prep_in_maps(inputs):
    import ml_dtypes
    bf = ml_dtypes.bfloat16

    x = np.asarray(inputs["x"], np.float32)
    Wq = np.asarray(inputs["Wq"], np.float32)
    Wk = np.asarray(inputs["Wk"], np.float32)
    Wv = np.asarray(inputs["Wv"], np.float32)
    Wo = np.asarray(inputs["Wo"], np.float32)
    W1 = np.asarray(inputs["W1"], np.float32)
    W2 = np.asarray(inputs["W2"], np.float32)
    bq = np.asarray(inputs["bq"], np.float32)
    bk = np.asarray(inputs["bk"], np.float32)
    bv = np.asarray(inputs["bv"], np.float32)
    bo = np.asarray(inputs["bo"], np.float32)
    b1 = np.asarray(inputs["b1"], np.float32)
    b2 = np.asarray(inputs["b2"], np.float32)
    g1 = np.asarray(inputs["g1"], np.float32)
    be1 = np.asarray(inputs["be1"], np.float32)
    g2 = np.asarray(inputs["g2"], np.float32)
    be2 = np.asarray(inputs["be2"], np.float32)
    a1 = np.asarray(inputs["a1"], np.float64)
    br1 = np.asarray(inputs["br1"], np.float64)
    a2 = np.asarray(inputs["a2"], np.float64)
    br2 = np.asarray(inputs["br2"], np.float64)
    rel_emb = np.asarray(inputs["rel_emb"], np.float32)
    res_scale = np.float32(np.clip(np.asarray(inputs["res_scale"],
                                              np.float32), 0.2, 1.0))
    mask = np.asarray(inputs["casual_mask"], bool)

    # device path requires the canonical causal mask and zero V-bias
    bvp = Wv @ be1 + bv
    if not np.array_equal(mask, np.tril(np.ones((T, T), bool))):
        raise ValueError("non-causal mask unsupported on device")
    if np.any(bvp != 0):
        raise ValueError("nonzero folded V bias unsupported on device")

    sc = np.float32(DH ** -0.5)
    wqT = np.ascontiguousarray(((Wq * g1[None, :]).T * sc).astype(bf))
    bqp = ((Wq @ be1 + bq) * sc).astype(np.float32)
    wkT = np.ascontiguousarray((Wk * g1[None, :]).T.astype(bf))
    bkp = (Wk @ be1 + bk).astype(np.float32)
    wvT = np.ascontiguousarray((Wv * g1[None, :]).T.astype(bf))
    woT = np.ascontiguousarray(Wo.T.astype(bf))
    bop = bo
    w1T = np.ascontiguousarray((W1 * g2[None, :]).T.astype(bf))
    b1p = (W1 @ be2 + b1).astype(np.float32)
    w2T = np.ascontiguousarray(W2.T.astype(bf))
    b2p = b2

    # shifted LN polynomials (fold the +eps into the coefficients)
    eps = 1e-5

    def shift(c0, c1, c2):
        return (np.float32(c0 + c1 * eps + c2 * eps * eps),
                np.float32(c1 + 2 * c2 * eps), np.float32(c2))

    p1 = shift(*a1)
    sb1 = _softplus10(br1)
    q1 = shift(*sb1)
    p2 = shift(*a2)
    sb2 = _softplus10(br2)
    q2 = shift(*sb2)

    consts = np.zeros((128, NCOL_CONSTS), np.float32)
    consts[:, 0] = res_scale
    consts[:, 1:4] = p1
    consts[:, 4:7] = q1
    consts[:, 7:10] = p2
    consts[:, 10:13] = q2
    consts[:, 13] = 1e-6

    # Toeplitz bias+mask slabs, one per q-block
    # band_full[h, i], i = (k - q) + 2047: rel bias if k<=q else -1e9
    i_ax = np.arange(4096)
    kq = i_ax - 2047
    buck = np.clip(kq, -MAX_REL + 1, MAX_REL - 1) + (MAX_REL - 1)
    band = rel_emb[buck, :].T.astype(np.float32)  # [H, 4096]
    band[:, kq > 0] = -1e9
    pp = np.arange(128)[:, None]
    mm = np.arange(SLAB_W)[None, :]
    slabs, mepss = {}, {}
    for qb in range(4):
        q0 = qb * QT
        idx = pp - mm + 3967 - q0
        slabs[qb] = np.ascontiguousarray(band[:, idx].astype(bf))
        mepss[qb] = np.ascontiguousarray(
            np.where(idx <= 2047, np.float32(1e-6), np.float32(0.0)
                     ).astype(bf))

    def colpack(v):
        return np.ascontiguousarray(v.reshape(-1, 128).T.astype(np.float32))

    bq_c, bk_c, bo_c = colpack(bqp), colpack(bkp), colpack(bop)
    b1_c, b2_c = colpack(b1p), colpack(b2p)

    xT = {b_i: np.ascontiguousarray(x[b_i].T.astype(bf)) for b_i in range(B)}

    in_maps = []
    for c in range(N_CORES):
        b_i, qb = c // 4, c % 4
        in_maps.append({
            "xT": xT[b_i],
            "xqT": np.ascontiguousarray(xT[b_i][:, qb * QT:(qb + 1) * QT]),
            "wqT": wqT, "wkT": wkT, "wvT": wvT, "woT": woT,
            "w1T": w1T, "w2T": w2T,
            "slab": slabs[qb], "meps": mepss[qb],
            "bqc": bq_c, "bkc": bk_c, "boc": bo_c,
            "b1c": b1_c, "b2c": b2_c,
            "consts": consts,
        })
    return in_maps


def _assemble(results):
    out = np.empty((B, T, D_MODEL), np.float32)
    for c in range(N_CORES):
        b_i, qb = c // 4, c % 4
        out[b_i, qb * QT:(qb + 1) * QT, :] = results[c]["out"].T
    return out


def _fix_first_rows(out, inputs, nrows=32):
    """Recompute the first `nrows` query rows of each batch exactly.

    Early rows have tiny attention mass (sum of relu'd scores ~ q), so the
    relu sign of near-zero bf16 scores can materially change their weights.
    Causality means row q depends only on tokens <= q, so an exact fp32
    recompute of the first rows costs microseconds.
    """
    sub = {k: v for k, v in inputs.items()}
    sub["x"] = np.asarray(inputs["x"], np.float32)[:, :nrows, :]
    sub["casual_mask"] = np.asarray(inputs["casual_mask"],
                                    bool)[:nrows, :nrows]
    global T
    t_save = T
    try:
        T = nrows
        out[:, :nrows, :] = _kernel_numpy(**sub)
    finally:
        T = t_save
    return out


def _get_nc():
    if "nc" not in _STATE:
        _ensure_hooks()
        _STATE["nc"] = _build_nc()
    return _STATE["nc"]


def kernel(**inputs):
    inputs = {k: np.asarray(v) for k, v in inputs.items()}
    try:
        from concourse.bass_utils import run_bass_kernel_spmd
        nc = _get_nc()
        in_maps = _prep_in_maps(inputs)
        res = run_bass_kernel_spmd(nc, in_maps, core_ids=list(range(N_CORES)))
        out = _assemble(res.results)
        if not np.all(np.isfinite(out)):
            raise ValueError("non-finite device output")
        return _fix_first_rows(out, inputs)
    except Exception as e:
        import traceback
        traceback.print_exc()
        print(f"device path failed ({e!r}); using numpy fallback",
              file=sys.stderr)
        return _kernel_numpy(**inputs)
